# revision 18
# baseline (speedup 1.0000x reference)
"""LSTM cell (B=4096, I=H=1024, fp32) on 8 Trainium2 NeuronCores.

Strategy
--------
Sharding: 2-D -- batch split 4 ways x hidden split 2 ways (8 cores).
Per core: gates^T = Wcat^T_chunk @ xh^T in the transposed [hidden,
batch] layout (host does all transposes/packing in numpy), so every
DMA is contiguous per partition.

Schedule: the per-core GEMM ([K=2048] x [M=2048] x [N=1024]) is tiled
as 4 row-blocks (hm) x 4 gates x 2 batch halves, with the contraction
(kc) innermost-but-one so the PE consumes each freshly-arrived xh
chunk for 8 matmuls at once (hm0 runs both batch halves interleaved
across all 8 PSUM banks).  That keeps TensorE dense from ~2us in,
so the HAM clock warms early and stays warm.  Weight tiles stream on
the scalar-engine HWDGE queue (FIFO => arrival follows program order:
w0 chunks first, then w1; w2/w3 are emitted later, paced by PSUM-pool
reuse), while xh/c/stores ride the sync-engine queue.  The last
epilogue is gate-staggered (f, c~, i computed before o's k-loop ends)
to shorten the kernel tail.

Dtype: DT_MODE="fp16" runs the GEMM with float16 operands (fp32 PSUM
accumulate) — same 1 cycle/row PE rate and 2-byte DMA as bf16, but
L2 rel err ~1.7e-3 (bf16: ~1.3e-2, f32r: same error at 2x the bytes).
"""
import numpy as np
import ml_dtypes
import concourse.bacc as bacc
import concourse.mybir as mybir
import concourse.tile as tile
from concourse.bass_utils import run_bass_kernel_spmd

B, I, H = 4096, 1024, 1024
BS, HS = 4, 2          # batch shards x hidden shards = 8 cores
BC = B // BS           # 1024 batch rows per core
GC = H // HS           # 512 hidden rows per gate per core
K = I + H              # 2048 contraction
KT = K // 128          # 16 k-chunks
QT = GC // 128         # 4 hidden row-blocks (hm) per core
NT = 2                 # batch halves of 512 (PSUM bank = 512 fp32)

DT_MODE = "fp16"       # "fp16" | "bf16" | "f32r"

f32 = mybir.dt.float32
f32r = mybir.dt.float32r
bf16 = mybir.dt.bfloat16
AF = mybir.ActivationFunctionType
BF = ml_dtypes.bfloat16


def build_nc(mode=DT_MODE):
    if mode == "bf16":
        io_dt = mm_dt = ew_dt = bf16
    elif mode == "fp16":
        io_dt = mm_dt = ew_dt = mybir.dt.float16
    else:  # f32r
        io_dt, mm_dt, ew_dt = f32, f32r, f32

    def cast(ap):
        return ap if mode != "f32r" else ap.bitcast(f32r)

    nc = bacc.Bacc(None, target_bir_lowering=False)
    xh_d = nc.dram_tensor("xh_t", [K, BC], io_dt, kind="ExternalInput")
    wp_d = nc.dram_tensor("wp", [QT, KT, 128, 512], io_dt, kind="ExternalInput")
    c_d = nc.dram_tensor("c_t", [GC, BC], io_dt, kind="ExternalInput")
    b_d = nc.dram_tensor("bias", [128, 16], f32, kind="ExternalInput")
    ht_d = nc.dram_tensor("ht_t", [GC, BC], f32, kind="ExternalOutput")

    with tile.TileContext(nc) as tc:
        with (
            tc.tile_pool(name="xpool", bufs=1) as xpool,
            tc.tile_pool(name="wpool", bufs=2) as wpool,
            tc.tile_pool(name="cpool", bufs=1) as cpool,
            tc.tile_pool(name="gpool", bufs=2) as gpool,
            tc.tile_pool(name="epool", bufs=2) as epool,
            tc.tile_pool(name="opool", bufs=2) as opool,
            tc.tile_pool(name="psum", bufs=2, space="PSUM") as psum,
        ):
            xh_r = xh_d.rearrange("(k p) b -> k p b", p=128)
            c_r = c_d.rearrange("(q p) b -> q p b", p=128)
            w_bulk = wp_d.rearrange("q k p m -> q p k m")

            # xh and hm0-weight chunks stream interleaved across BOTH HWDGE
            # queues (each queue is FIFO, so arrival follows emission
            # order); first chunks are single-kc so the first matmul can
            # start ASAP.  Phase-1 consumption needs xh[k] + w0[k] at
            # ~1.7us/kc; splitting across the queues halves each backlog.
            xh_sb = xpool.tile([128, KT, BC], mm_dt, tag="xh", name="xh_sb")
            w_sb = [None] * QT
            w_sb[0] = wpool.tile([128, KT, 512], mm_dt, tag="wg", name="w_sb")

            def load_xh(eng, lo, hi):
                eng.dma_start(xh_sb[:, lo:hi, :],
                              cast(xh_r[lo:hi].rearrange("k p b -> p k b")))

            def load_w0(eng, lo, hi):
                eng.dma_start(w_sb[0][:, lo:hi, :],
                              cast(wp_d[0, lo:hi].rearrange("k p m -> p k m")))

            pairs = [(j, j + 1) for j in range(8)] + \
                    [(2 * j, 2 * j + 2) for j in range(4, KT // 2)]
            for n, (lo, hi) in enumerate(pairs):
                if n < 9 or n % 2 == 1:
                    load_xh(nc.sync, lo, hi)
                    load_w0(nc.scalar, lo, hi)
                else:
                    load_xh(nc.scalar, lo, hi)
                    load_w0(nc.sync, lo, hi)
            # cell state, resident (tiny)
            c_sb = []
            for q in range(QT):
                t = cpool.tile([128, BC], ew_dt, tag=f"c{q}", name="c_sb")
                nc.sync.dma_start(t[:], c_r[q])
                c_sb.append(t)
            bias_sb = cpool.tile([128, 16], f32, tag="bias", name="bias_sb")
            nc.sync.dma_start(bias_sb[:], b_d[:, :])

            w_sb[1] = wpool.tile([128, KT, 512], mm_dt, tag="wg", name="w_sb")
            nc.scalar.dma_start(w_sb[1][:], cast(w_bulk[1]))

            # HAM warm-up: dense dummy matmuls on a zeroed tile while the
            # first xh/weight chunks stream in, so the PE clock gate is at
            # 8/8 when the real stream starts (saves ~13 half-rate MMs).
            # The dummy activation pre-loads the sigmoid/tanh table set
            # (~1.3us) that otherwise stalls the first PSUM evacuation.
            warm = cpool.tile([128, 512], mm_dt, tag="warm", name="warm")
            nc.vector.memset(warm[:], 0)
            warm2 = cpool.tile([128, 4], ew_dt, tag="warma", name="warm2")
            nc.scalar.activation(warm2[:], warm[:, 0:4], AF.Sigmoid)
            warm_acc = psum.tile([128, 512], f32, tag="a0", name="warm_acc")
            for _ in range(8):
                nc.tensor.matmul(warm_acc[:], warm[:, 0:128], warm[:],
                                 start=True, stop=True)

            def epilogue(hm, nh, accs, split_o=False):
                gts = []
                for g in range(4):
                    func = AF.Tanh if g == 2 else AF.Sigmoid
                    gt = gpool.tile([128, 512], ew_dt, tag=f"g{g}", name="gt")
                    bcol = hm * 4 + g
                    if g == 3 and split_o:
                        for s in range(2):
                            cs = slice(s * 256, (s + 1) * 256)
                            nc.scalar.activation(gt[:, cs], accs[g][:, cs],
                                                 func,
                                                 bias=bias_sb[:, bcol:bcol + 1])
                    else:
                        nc.scalar.activation(gt[:], accs[g][:], func,
                                             bias=bias_sb[:, bcol:bcol + 1])
                    gts.append(gt)
                cs_ = c_sb[hm][:, nh * 512:(nh + 1) * 512]
                t1 = epool.tile([128, 512], ew_dt, tag="t1", name="t1")
                nc.vector.tensor_mul(t1[:], gts[0][:], cs_)
                t2 = epool.tile([128, 512], ew_dt, tag="t2", name="t2")
                nc.vector.tensor_mul(t2[:], gts[1][:], gts[2][:])
                cn = epool.tile([128, 512], ew_dt, tag="cn", name="cn")
                nc.vector.tensor_add(cn[:], t1[:], t2[:])
                tc_ = epool.tile([128, 512], ew_dt, tag="tc", name="tc_")
                nc.scalar.activation(tc_[:], cn[:], AF.Tanh)
                ho = opool.tile([128, 512], f32, tag="ho", name="ho")
                rows = slice(hm * 128, (hm + 1) * 128)
                if split_o:
                    # pipeline the tail: store each 256-col chunk as soon
                    # as its final multiply lands
                    for s in range(2):
                        cs = slice(s * 256, (s + 1) * 256)
                        nc.vector.tensor_mul(ho[:, cs], gts[3][:, cs],
                                             tc_[:, cs])
                        nc.sync.dma_start(
                            ht_d[rows, nh * 512 + s * 256:
                                 nh * 512 + (s + 1) * 256],
                            ho[:, cs])
                else:
                    nc.vector.tensor_mul(ho[:], gts[3][:], tc_[:])
                    nc.sync.dma_start(
                        ht_d[rows, nh * 512:(nh + 1) * 512], ho[:])

            def mm(accs_g, hm, kc, g, nh, start, stop):
                nc.tensor.matmul(
                    accs_g[:],
                    w_sb[hm][:, kc, g * 128:(g + 1) * 128],
                    xh_sb[:, kc, nh * 512:(nh + 1) * 512],
                    start=start, stop=stop)

            # ---- hm0: both batch halves interleaved over all 8 banks ----
            accs0 = [[psum.tile([128, 512], f32, tag=f"a{g}", name=f"a{g}")
                      for g in range(4)] for nh in range(2)]
            for kc in range(KT):
                for nh in range(2):
                    for g in range(4):
                        mm(accs0[nh][g], 0, kc, g, nh, kc == 0, kc == KT - 1)
            for nh in range(2):
                epilogue(0, nh, accs0[nh])

            # w2 paced: emitted after hm0 epilogues; pool reuse makes it
            # wait for hm0's last matmul before the transfer starts.
            w_sb[2] = wpool.tile([128, KT, 512], mm_dt, tag="wg", name="w_sb")
            nc.scalar.dma_start(w_sb[2][:], cast(w_bulk[2]))

            # ---- hm1, hm2: per batch-half (4 banks each, double-buffered)
            for hm in (1, 2):
                for nh in range(2):
                    accs = [psum.tile([128, 512], f32, tag=f"a{g}",
                                      name=f"a{g}") for g in range(4)]
                    for kc in range(KT):
                        for g in range(4):
                            mm(accs[g], hm, kc, g, nh, kc == 0, kc == KT - 1)
                    epilogue(hm, nh, accs)
                if hm == 1:
                    w_sb[3] = wpool.tile([128, KT, 512], mm_dt, tag="wg", name="w_sb")
                    nc.scalar.dma_start(w_sb[3][:], cast(w_bulk[3]))

            # ---- hm3: n0 normal; n1 gate-staggered to shorten the tail
            accs = [psum.tile([128, 512], f32, tag=f"a{g}", name=f"a{g}")
                    for g in range(4)]
            for kc in range(KT):
                for g in range(4):
                    mm(accs[g], 3, kc, g, 0, kc == 0, kc == KT - 1)
            epilogue(3, 0, accs)

            # f, c~, i accumulate first (their epilogue chain runs during
            # o's k-loops); o is split into two sequential 256-col k-runs
            # in SEPARATE banks (start=True clears a whole bank) so the
            # first half's act+mul+store hide under the second half's MMs.
            accs31 = {}
            for g in (0, 2, 1):
                accs31[g] = psum.tile([128, 512], f32, tag=f"a{g}",
                                      name=f"a{g}")
                for kc in range(KT):
                    mm(accs31[g], 3, kc, g, 1, kc == 0, kc == KT - 1)
            o_half = [psum.tile([128, 512], f32, tag="a3", name="o0"),
                      psum.tile([128, 512], f32, tag="a0", name="o1")]
            for s in range(2):
                for kc in range(KT):
                    nc.tensor.matmul(
                        o_half[s][:, 0:256],
                        w_sb[3][:, kc, 3 * 128:4 * 128],
                        xh_sb[:, kc, 512 + s * 256:512 + (s + 1) * 256],
                        start=kc == 0, stop=kc == KT - 1)

            gts = {}
            for g in (0, 2, 1):
                func = AF.Tanh if g == 2 else AF.Sigmoid
                gt = gpool.tile([128, 512], ew_dt, tag=f"g{g}", name="gt")
                nc.scalar.activation(gt[:], accs31[g][:], func,
                                     bias=bias_sb[:, 12 + g:13 + g])
                gts[g] = gt
            cs_ = c_sb[3][:, 512:1024]
            t1 = epool.tile([128, 512], ew_dt, tag="t1", name="t1")
            nc.vector.tensor_mul(t1[:], gts[0][:], cs_)
            t2 = epool.tile([128, 512], ew_dt, tag="t2", name="t2")
            nc.vector.tensor_mul(t2[:], gts[1][:], gts[2][:])
            cn = epool.tile([128, 512], ew_dt, tag="cn", name="cn")
            nc.vector.tensor_add(cn[:], t1[:], t2[:])
            tc_ = epool.tile([128, 512], ew_dt, tag="tc", name="tc_")
            nc.scalar.activation(tc_[:], cn[:], AF.Tanh)
            ho = opool.tile([128, 512], f32, tag="ho", name="ho")
            for s in range(2):
                cs = slice(s * 256, (s + 1) * 256)
                og = gpool.tile([128, 256], ew_dt, tag="g3", name="og")
                nc.scalar.activation(og[:], o_half[s][:, 0:256], AF.Sigmoid,
                                     bias=bias_sb[:, 15:16])
                nc.vector.tensor_mul(ho[:, cs], og[:], tc_[:, cs])
                nc.sync.dma_start(
                    ht_d[3 * 128:4 * 128, 512 + s * 256:512 + (s + 1) * 256],
                    ho[:, cs])
    nc.compile()
    return nc


_NC_CACHE = {}


def _get_nc(mode=DT_MODE):
    if mode not in _NC_CACHE:
        _NC_CACHE[mode] = build_nc(mode)
    return _NC_CACHE[mode]


def _make_in_maps(inputs, mode=DT_MODE):
    np_dt = {"bf16": BF, "fp16": np.float16, "f32r": np.float32}[mode]
    f = lambda name: np.ascontiguousarray(np.asarray(inputs[name],
                                                     dtype=np.float32))
    xh = np.concatenate([f("x_t"), f("h_prev")], axis=1)            # [B, K]
    Wfull = np.concatenate([
        np.concatenate([f("W_f"), f("W_i"), f("W_c"), f("W_o")], axis=1),
        np.concatenate([f("U_f"), f("U_i"), f("U_c"), f("U_o")], axis=1),
    ], axis=0)                                                      # [K, 4H]
    bias_full = np.concatenate([f("b_f"), f("b_i"), f("b_c"), f("b_o")])
    c_prev = f("c_prev")

    in_maps = []
    for core in range(BS * HS):
        bi, hi = divmod(core, HS)
        cols = np.concatenate(
            [np.arange(g * H + hi * GC, g * H + (hi + 1) * GC)
             for g in range(4)])
        Wc = Wfull[:, cols]                                         # [K, 2048]
        # wp[hm, kc, p, g*128+mm] = Wc[kc*128+p, g*512+hm*128+mm]
        wp = (Wc.reshape(KT, 128, 4, QT, 128)
                .transpose(3, 0, 1, 2, 4)
                .reshape(QT, KT, 128, 512))
        # [128, 16]: column hm*4+g holds that gate-tile's per-row bias
        bias_pk = (bias_full[cols].reshape(4, QT, 128)
                   .transpose(2, 1, 0).reshape(128, 16))
        im = {
            "xh_t": np.ascontiguousarray(
                xh[bi * BC:(bi + 1) * BC, :].T.astype(np_dt)),
            "wp": np.ascontiguousarray(wp.astype(np_dt)),
            "c_t": np.ascontiguousarray(
                c_prev[bi * BC:(bi + 1) * BC,
                       hi * GC:(hi + 1) * GC].T.astype(np_dt)),
            "bias": np.ascontiguousarray(bias_pk),
        }
        in_maps.append(im)
    return in_maps


def _run(inputs, mode=DT_MODE, **spmd_kwargs):
    nc = _get_nc(mode)
    in_maps = _make_in_maps(inputs, mode)
    res = run_bass_kernel_spmd(nc, in_maps, core_ids=list(range(BS * HS)),
                               **spmd_kwargs)
    h_t = np.empty((B, H), dtype=np.float32)
    for core in range(BS * HS):
        bi, hi = divmod(core, HS)
        h_t[bi * BC:(bi + 1) * BC,
            hi * GC:(hi + 1) * GC] = res.results[core]["ht_t"].T
    return h_t, res


def kernel(**inputs) -> np.ndarray:
    h_t, _ = _run(inputs)
    return h_t


# revision 19
# speedup vs baseline: 1.0123x; 1.0123x over previous
"""LSTM cell (B=4096, I=H=1024, fp32) on 8 Trainium2 NeuronCores.

Strategy
--------
Sharding: 2-D -- batch split 4 ways x hidden split 2 ways (8 cores).
Per core: gates^T = Wcat^T_chunk @ xh^T in the transposed [hidden,
batch] layout (host does all transposes/packing in numpy), so every
DMA is contiguous per partition.

Schedule: the per-core GEMM ([K=2048] x [M=2048] x [N=1024]) is tiled
as 4 row-blocks (hm) x 4 gates x 2 batch halves, with the contraction
(kc) innermost-but-one so the PE consumes each freshly-arrived xh
chunk for 8 matmuls at once (hm0 runs both batch halves interleaved
across all 8 PSUM banks).  That keeps TensorE dense from ~2us in,
so the HAM clock warms early and stays warm.  Weight tiles stream on
the scalar-engine HWDGE queue (FIFO => arrival follows program order:
w0 chunks first, then w1; w2/w3 are emitted later, paced by PSUM-pool
reuse), while xh/c/stores ride the sync-engine queue.  The last
epilogue is gate-staggered (f, c~, i computed before o's k-loop ends)
to shorten the kernel tail.

Dtype: DT_MODE="fp16" runs the GEMM with float16 operands (fp32 PSUM
accumulate) — same 1 cycle/row PE rate and 2-byte DMA as bf16, but
L2 rel err ~1.7e-3 (bf16: ~1.3e-2, f32r: same error at 2x the bytes).
"""
import numpy as np
import ml_dtypes
import concourse.bacc as bacc
import concourse.mybir as mybir
import concourse.tile as tile
from concourse.bass_utils import run_bass_kernel_spmd

B, I, H = 4096, 1024, 1024
BS, HS = 4, 2          # batch shards x hidden shards = 8 cores
BC = B // BS           # 1024 batch rows per core
GC = H // HS           # 512 hidden rows per gate per core
K = I + H              # 2048 contraction
KT = K // 128          # 16 k-chunks
QT = GC // 128         # 4 hidden row-blocks (hm) per core
NT = 2                 # batch halves of 512 (PSUM bank = 512 fp32)

DT_MODE = "fp16"       # "fp16" | "bf16" | "f32r"

f32 = mybir.dt.float32
f32r = mybir.dt.float32r
bf16 = mybir.dt.bfloat16
AF = mybir.ActivationFunctionType
BF = ml_dtypes.bfloat16


def build_nc(mode=DT_MODE):
    if mode == "bf16":
        io_dt = mm_dt = ew_dt = bf16
    elif mode == "fp16":
        io_dt = mm_dt = ew_dt = mybir.dt.float16
    else:  # f32r
        io_dt, mm_dt, ew_dt = f32, f32r, f32

    def cast(ap):
        return ap if mode != "f32r" else ap.bitcast(f32r)

    nc = bacc.Bacc(None, target_bir_lowering=False)
    xh_d = nc.dram_tensor("xh_t", [K, BC], io_dt, kind="ExternalInput")
    wp_d = nc.dram_tensor("wp", [QT, KT, 128, 512], io_dt, kind="ExternalInput")
    c_d = nc.dram_tensor("c_t", [GC, BC], io_dt, kind="ExternalInput")
    b_d = nc.dram_tensor("bias", [128, 16], f32, kind="ExternalInput")
    ht_d = nc.dram_tensor("ht_t", [GC, BC], f32, kind="ExternalOutput")

    with tile.TileContext(nc) as tc:
        with (
            tc.tile_pool(name="xpool", bufs=1) as xpool,
            tc.tile_pool(name="wpool", bufs=2) as wpool,
            tc.tile_pool(name="cpool", bufs=1) as cpool,
            tc.tile_pool(name="gpool", bufs=2) as gpool,
            tc.tile_pool(name="epool", bufs=2) as epool,
            tc.tile_pool(name="opool", bufs=2) as opool,
            tc.tile_pool(name="psum", bufs=2, space="PSUM") as psum,
        ):
            xh_r = xh_d.rearrange("(k p) b -> k p b", p=128)
            c_r = c_d.rearrange("(q p) b -> q p b", p=128)
            w_bulk = wp_d.rearrange("q k p m -> q p k m")

            # xh and hm0-weight chunks stream interleaved across BOTH HWDGE
            # queues (each queue is FIFO, so arrival follows emission
            # order); first chunks are single-kc so the first matmul can
            # start ASAP.  Phase-1 consumption needs xh[k] + w0[k] at
            # ~1.7us/kc; splitting across the queues halves each backlog.
            xh_sb = xpool.tile([128, KT, BC], mm_dt, tag="xh", name="xh_sb")
            w_sb = [None] * QT
            w_sb[0] = wpool.tile([128, KT, 512], mm_dt, tag="wg", name="w_sb")

            def load_xh(eng, lo, hi):
                eng.dma_start(xh_sb[:, lo:hi, :],
                              cast(xh_r[lo:hi].rearrange("k p b -> p k b")))

            def load_w0(eng, lo, hi):
                eng.dma_start(w_sb[0][:, lo:hi, :],
                              cast(wp_d[0, lo:hi].rearrange("k p m -> p k m")))

            pairs = [(j, j + 1) for j in range(8)] + \
                    [(2 * j, 2 * j + 2) for j in range(4, KT // 2)]
            for n, (lo, hi) in enumerate(pairs):
                if n < 9 or n % 2 == 1:
                    load_xh(nc.sync, lo, hi)
                    load_w0(nc.scalar, lo, hi)
                else:
                    load_xh(nc.scalar, lo, hi)
                    load_w0(nc.sync, lo, hi)
            # cell state, resident (tiny)
            c_sb = []
            for q in range(QT):
                t = cpool.tile([128, BC], ew_dt, tag=f"c{q}", name="c_sb")
                nc.sync.dma_start(t[:], c_r[q])
                c_sb.append(t)
            bias_sb = cpool.tile([128, 16], f32, tag="bias", name="bias_sb")
            nc.sync.dma_start(bias_sb[:], b_d[:, :])

            w_sb[1] = wpool.tile([128, KT, 512], mm_dt, tag="wg", name="w_sb")
            nc.scalar.dma_start(w_sb[1][:], cast(w_bulk[1]))

            # HAM warm-up: dense dummy matmuls on a zeroed tile while the
            # first xh/weight chunks stream in, so the PE clock gate is at
            # 8/8 when the real stream starts (saves ~13 half-rate MMs).
            # The dummy activation pre-loads the sigmoid/tanh table set
            # (~1.3us) that otherwise stalls the first PSUM evacuation.
            warm = cpool.tile([128, 512], mm_dt, tag="warm", name="warm")
            nc.vector.memset(warm[:], 0)
            warm2 = cpool.tile([128, 4], ew_dt, tag="warma", name="warm2")
            nc.scalar.activation(warm2[:], warm[:, 0:4], AF.Sigmoid)
            warm_acc = psum.tile([128, 512], f32, tag="a0", name="warm_acc")
            for _ in range(9):
                nc.tensor.matmul(warm_acc[:], warm[:, 0:128], warm[:],
                                 start=True, stop=True)

            def epilogue(hm, nh, accs, split_o=False):
                gts = []
                for g in range(4):
                    func = AF.Tanh if g == 2 else AF.Sigmoid
                    gt = gpool.tile([128, 512], ew_dt, tag=f"g{g}", name="gt")
                    bcol = hm * 4 + g
                    if g == 3 and split_o:
                        for s in range(2):
                            cs = slice(s * 256, (s + 1) * 256)
                            nc.scalar.activation(gt[:, cs], accs[g][:, cs],
                                                 func,
                                                 bias=bias_sb[:, bcol:bcol + 1])
                    else:
                        nc.scalar.activation(gt[:], accs[g][:], func,
                                             bias=bias_sb[:, bcol:bcol + 1])
                    gts.append(gt)
                cs_ = c_sb[hm][:, nh * 512:(nh + 1) * 512]
                t1 = epool.tile([128, 512], ew_dt, tag="t1", name="t1")
                nc.vector.tensor_mul(t1[:], gts[0][:], cs_)
                t2 = epool.tile([128, 512], ew_dt, tag="t2", name="t2")
                nc.vector.tensor_mul(t2[:], gts[1][:], gts[2][:])
                cn = epool.tile([128, 512], ew_dt, tag="cn", name="cn")
                nc.vector.tensor_add(cn[:], t1[:], t2[:])
                tc_ = epool.tile([128, 512], ew_dt, tag="tc", name="tc_")
                nc.scalar.activation(tc_[:], cn[:], AF.Tanh)
                ho = opool.tile([128, 512], f32, tag="ho", name="ho")
                rows = slice(hm * 128, (hm + 1) * 128)
                if split_o:
                    # pipeline the tail: store each 256-col chunk as soon
                    # as its final multiply lands
                    for s in range(2):
                        cs = slice(s * 256, (s + 1) * 256)
                        nc.vector.tensor_mul(ho[:, cs], gts[3][:, cs],
                                             tc_[:, cs])
                        nc.sync.dma_start(
                            ht_d[rows, nh * 512 + s * 256:
                                 nh * 512 + (s + 1) * 256],
                            ho[:, cs])
                else:
                    nc.vector.tensor_mul(ho[:], gts[3][:], tc_[:])
                    nc.sync.dma_start(
                        ht_d[rows, nh * 512:(nh + 1) * 512], ho[:])

            def mm(accs_g, hm, kc, g, nh, start, stop):
                nc.tensor.matmul(
                    accs_g[:],
                    w_sb[hm][:, kc, g * 128:(g + 1) * 128],
                    xh_sb[:, kc, nh * 512:(nh + 1) * 512],
                    start=start, stop=stop)

            # ---- hm0: both batch halves interleaved over all 8 banks ----
            accs0 = [[psum.tile([128, 512], f32, tag=f"a{g}", name=f"a{g}")
                      for g in range(4)] for nh in range(2)]
            for kc in range(KT):
                for nh in range(2):
                    for g in range(4):
                        mm(accs0[nh][g], 0, kc, g, nh, kc == 0, kc == KT - 1)
            for nh in range(2):
                epilogue(0, nh, accs0[nh])

            # w2 paced: emitted after hm0 epilogues; pool reuse makes it
            # wait for hm0's last matmul before the transfer starts.
            w_sb[2] = wpool.tile([128, KT, 512], mm_dt, tag="wg", name="w_sb")
            nc.scalar.dma_start(w_sb[2][:], cast(w_bulk[2]))

            # ---- hm1, hm2: per batch-half (4 banks each, double-buffered)
            for hm in (1, 2):
                for nh in range(2):
                    accs = [psum.tile([128, 512], f32, tag=f"a{g}",
                                      name=f"a{g}") for g in range(4)]
                    for kc in range(KT):
                        for g in range(4):
                            mm(accs[g], hm, kc, g, nh, kc == 0, kc == KT - 1)
                    epilogue(hm, nh, accs)
                if hm == 1:
                    w_sb[3] = wpool.tile([128, KT, 512], mm_dt, tag="wg", name="w_sb")
                    nc.scalar.dma_start(w_sb[3][:], cast(w_bulk[3]))

            # ---- hm3: n0 normal; n1 gate-staggered to shorten the tail
            accs = [psum.tile([128, 512], f32, tag=f"a{g}", name=f"a{g}")
                    for g in range(4)]
            for kc in range(KT):
                for g in range(4):
                    mm(accs[g], 3, kc, g, 0, kc == 0, kc == KT - 1)
            epilogue(3, 0, accs)

            # f, c~, i accumulate first (their epilogue chain runs during
            # o's k-loops); o is split into two sequential 256-col k-runs
            # in SEPARATE banks (start=True clears a whole bank) so the
            # first half's act+mul+store hide under the second half's MMs.
            accs31 = {}
            for g in (0, 2, 1):
                accs31[g] = psum.tile([128, 512], f32, tag=f"a{g}",
                                      name=f"a{g}")
                for kc in range(KT):
                    mm(accs31[g], 3, kc, g, 1, kc == 0, kc == KT - 1)
            o_half = [psum.tile([128, 512], f32, tag="a3", name="o0"),
                      psum.tile([128, 512], f32, tag="a0", name="o1")]
            for s in range(2):
                for kc in range(KT):
                    nc.tensor.matmul(
                        o_half[s][:, 0:256],
                        w_sb[3][:, kc, 3 * 128:4 * 128],
                        xh_sb[:, kc, 512 + s * 256:512 + (s + 1) * 256],
                        start=kc == 0, stop=kc == KT - 1)

            gts = {}
            for g in (0, 2, 1):
                func = AF.Tanh if g == 2 else AF.Sigmoid
                gt = gpool.tile([128, 512], ew_dt, tag=f"g{g}", name="gt")
                nc.scalar.activation(gt[:], accs31[g][:], func,
                                     bias=bias_sb[:, 12 + g:13 + g])
                gts[g] = gt
            cs_ = c_sb[3][:, 512:1024]
            t1 = epool.tile([128, 512], ew_dt, tag="t1", name="t1")
            nc.vector.tensor_mul(t1[:], gts[0][:], cs_)
            t2 = epool.tile([128, 512], ew_dt, tag="t2", name="t2")
            nc.vector.tensor_mul(t2[:], gts[1][:], gts[2][:])
            cn = epool.tile([128, 512], ew_dt, tag="cn", name="cn")
            nc.vector.tensor_add(cn[:], t1[:], t2[:])
            tc_ = epool.tile([128, 512], ew_dt, tag="tc", name="tc_")
            nc.scalar.activation(tc_[:], cn[:], AF.Tanh)
            ho = opool.tile([128, 512], f32, tag="ho", name="ho")
            for s in range(2):
                cs = slice(s * 256, (s + 1) * 256)
                og = gpool.tile([128, 256], ew_dt, tag="g3", name="og")
                nc.scalar.activation(og[:], o_half[s][:, 0:256], AF.Sigmoid,
                                     bias=bias_sb[:, 15:16])
                nc.vector.tensor_mul(ho[:, cs], og[:], tc_[:, cs])
                nc.sync.dma_start(
                    ht_d[3 * 128:4 * 128, 512 + s * 256:512 + (s + 1) * 256],
                    ho[:, cs])
    nc.compile()
    return nc


_NC_CACHE = {}


def _get_nc(mode=DT_MODE):
    if mode not in _NC_CACHE:
        _NC_CACHE[mode] = build_nc(mode)
    return _NC_CACHE[mode]


def _make_in_maps(inputs, mode=DT_MODE):
    np_dt = {"bf16": BF, "fp16": np.float16, "f32r": np.float32}[mode]
    f = lambda name: np.ascontiguousarray(np.asarray(inputs[name],
                                                     dtype=np.float32))
    xh = np.concatenate([f("x_t"), f("h_prev")], axis=1)            # [B, K]
    Wfull = np.concatenate([
        np.concatenate([f("W_f"), f("W_i"), f("W_c"), f("W_o")], axis=1),
        np.concatenate([f("U_f"), f("U_i"), f("U_c"), f("U_o")], axis=1),
    ], axis=0)                                                      # [K, 4H]
    bias_full = np.concatenate([f("b_f"), f("b_i"), f("b_c"), f("b_o")])
    c_prev = f("c_prev")

    in_maps = []
    for core in range(BS * HS):
        bi, hi = divmod(core, HS)
        cols = np.concatenate(
            [np.arange(g * H + hi * GC, g * H + (hi + 1) * GC)
             for g in range(4)])
        Wc = Wfull[:, cols]                                         # [K, 2048]
        # wp[hm, kc, p, g*128+mm] = Wc[kc*128+p, g*512+hm*128+mm]
        wp = (Wc.reshape(KT, 128, 4, QT, 128)
                .transpose(3, 0, 1, 2, 4)
                .reshape(QT, KT, 128, 512))
        # [128, 16]: column hm*4+g holds that gate-tile's per-row bias
        bias_pk = (bias_full[cols].reshape(4, QT, 128)
                   .transpose(2, 1, 0).reshape(128, 16))
        im = {
            "xh_t": np.ascontiguousarray(
                xh[bi * BC:(bi + 1) * BC, :].T.astype(np_dt)),
            "wp": np.ascontiguousarray(wp.astype(np_dt)),
            "c_t": np.ascontiguousarray(
                c_prev[bi * BC:(bi + 1) * BC,
                       hi * GC:(hi + 1) * GC].T.astype(np_dt)),
            "bias": np.ascontiguousarray(bias_pk),
        }
        in_maps.append(im)
    return in_maps


def _run(inputs, mode=DT_MODE, **spmd_kwargs):
    nc = _get_nc(mode)
    in_maps = _make_in_maps(inputs, mode)
    res = run_bass_kernel_spmd(nc, in_maps, core_ids=list(range(BS * HS)),
                               **spmd_kwargs)
    h_t = np.empty((B, H), dtype=np.float32)
    for core in range(BS * HS):
        bi, hi = divmod(core, HS)
        h_t[bi * BC:(bi + 1) * BC,
            hi * GC:(hi + 1) * GC] = res.results[core]["ht_t"].T
    return h_t, res


def kernel(**inputs) -> np.ndarray:
    h_t, _ = _run(inputs)
    return h_t
